# revision 11
# baseline (speedup 1.0000x reference)
"""Self-contained Trainium2 kernel for nn_AutoregressiveGroupQuerySelfAttention.

Reference computation (B=2, S=2048, H=2048, 16 heads x 128 dim):
    q = (x @ Wq.T) -> heads; k likewise; v likewise
    q, k get RoPE; scores = (q @ k.T) * sqrt(D)   (faithful-to-source bug)
    causal softmax; ctx = attn @ v; out = ctx @ Wo.T

Sharding over 8 NeuronCores: core c = (b, g) with b = c // 4 (batch),
g = c % 4 (head-group of 4 heads = 512 hidden columns).  Each core computes
its head-group's context and a partial output  ctx_g @ Wo.T[g-rows, :];
the host sums the 4 partials per batch element.

Precision: logit path (q/k projections, rope, scores) in fp32r; value path
(v, P, Wo) in bf16; output stored bf16 and accumulated f32 on host.

Schedule: fully fused single phase.  The q/k projection for chunk c+1 is
split into 8 per-head "sub-waves" (16 accumulating matmuls each) that are
interleaved into chunk c's attention as PE filler, so the Tensor engine
never idles long enough for the HAM clock-gate to downclock it.  Weight
slices for the sub-waves are re-streamed from HBM each chunk (SBUF is too
small to hold them and all activation slabs), prefetched 3 sub-waves ahead.
The softmax reciprocal is broadcast across partitions via GpSimd instead of
a PE ones-matmul.  PSUM: 2 banks proj accumulators, 2 shared rope/transpose
banks, 4 attention banks.
"""
import numpy as np
import ml_dtypes

import concourse.bass as bass
import concourse.mybir as mybir
from concourse import bacc
from concourse.tile import TileContext
from concourse.bass_utils import run_bass_kernel_spmd

F32 = mybir.dt.float32
F32R = mybir.dt.float32r
BF16 = mybir.dt.bfloat16
AX = mybir.AxisListType
ALU = mybir.AluOpType
ACTF = mybir.ActivationFunctionType

B, S, H = 2, 2048, 2048
NUM_HEADS, D = 16, 128
N_CORES = 8
NH = 4                     # heads per core
HG = NH * D                # 512
ROPE_BASE = 10000.0

_NC_CACHE = {}
LAST_RESULTS = None        # BassKernelResults of the most recent run (for profiling)
TRACE = False


def _build(S_=S, H_=H, NH_=NH):
    DD = 128
    HG_ = NH_ * DD
    KT = H_ // 128
    SQT = S_ // 128
    CH = 512
    NCHUNK = S_ // CH

    nc = bacc.Bacc()
    xT = nc.declare_dram_parameter("xT", [H_, S_], F32R, isOutput=False)
    xbfT = nc.declare_dram_parameter("xbfT", [H_, S_], BF16, isOutput=False)
    wqT = nc.declare_dram_parameter("wqT", [H_, HG_], F32R, isOutput=False)
    wkT = nc.declare_dram_parameter("wkT", [H_, HG_], F32R, isOutput=False)
    wvT = nc.declare_dram_parameter("wvT", [H_, HG_], BF16, isOutput=False)
    woT = nc.declare_dram_parameter("woT", [HG_, H_], BF16, isOutput=False)
    cosT = nc.declare_dram_parameter("cosT", [128, S_], F32, isOutput=False)
    sinT = nc.declare_dram_parameter("sinT", [128, S_], F32, isOutput=False)
    rT = nc.declare_dram_parameter("rT", [128, 128], F32R, isOutput=False)
    ident = nc.declare_dram_parameter("ident", [128, 128], BF16, isOutput=False)
    mask = nc.declare_dram_parameter("mask", [128, 128], F32, isOutput=False)
    out = nc.declare_dram_parameter("out", [S_, H_], BF16, isOutput=True)

    wq3 = wqT.rearrange("(kt p) j -> p kt j", p=128)
    wk3 = wkT.rearrange("(kt p) j -> p kt j", p=128)
    wv3 = wvT.rearrange("(kt p) j -> p kt j", p=128)
    xT3 = xT.rearrange("(kt p) s -> p kt s", p=128)
    xbf3 = xbfT.rearrange("(kt p) s -> p kt s", p=128)

    from contextlib import ExitStack
    with TileContext(nc) as tc:
        with ExitStack() as _stk:
            def _pool(**kw):
                return _stk.enter_context(tc.tile_pool(**kw))
            slabp = _pool(name="slabs", bufs=1)
            qwp = _pool(name="qw", bufs=2)
            wstp = _pool(name="wst", bufs=2)
            wvop = _pool(name="wvo", bufs=1)
            xgp = _pool(name="xg", bufs=1)
            xvp = _pool(name="xv", bufs=2)
            tabp = _pool(name="tab", bufs=1)
            rawp = _pool(name="raw", bufs=2)
            t12p = _pool(name="t12", bufs=1)
            pslabp = _pool(name="pslab", bufs=2)
            ptp = _pool(name="ptpool", bufs=1)
            ctxp = _pool(name="ctxpool", bufs=1)
            ostp = _pool(name="ostage", bufs=2)
            statp = _pool(name="stats", bufs=3)
            rowp = _pool(name="rows", bufs=1)
            ppp = _pool(name="pp", bufs=2, space="PSUM")
            pshp = _pool(name="psh", bufs=2, space="PSUM")
            pap = _pool(name="pa", bufs=4, space="PSUM")
            # ---- persistent slabs
            krope = [slabp.tile([128, S_], F32R, tag=f"krope{h}", name=f"krope{h}")
                     for h in range(NH_)]
            vslab = slabp.tile([128, SQT * HG_], BF16, tag="vslab")
            ident_sb = slabp.tile([128, 128], BF16, tag="ident")
            nc.scalar.dma_start(out=ident_sb[:], in_=ident[:])
            mask_sb = slabp.tile([128, 128], F32, tag="mask")
            nc.scalar.dma_start(out=mask_sb[:], in_=mask[:])
            rT_sb = slabp.tile([128, 128], F32R, tag="rT")
            nc.scalar.dma_start(out=rT_sb[:], in_=rT[:])

            ctxT = [ctxp.tile([128, 2 * CH], BF16, tag=f"ctxT{h}", name=f"ctxT{h}")
                    for h in range(NH_)]

            # ---- proj sub-wave machinery ------------------------------------
            # SPECS: (qk, h, cc) — sub-wave computing q/k head h of chunk cc.
            SPECS = [(qk, h, cc) for cc in range(NCHUNK)
                     for qk in ("k", "q") for h in range(NH_)]
            wsl_tiles = {}

            def issue_wsl(i):
                if i >= len(SPECS):
                    return
                qk, h, cc = SPECS[i]
                t = wstp.tile([128, KT * 128], F32R, tag="wsl", name=f"w{qk}{h}c{cc}")
                w3 = wq3 if qk == "q" else wk3
                nc.scalar.dma_start(
                    out=t[:].rearrange("p (kt j) -> p kt j", kt=KT),
                    in_=w3[:, :, h * 128:(h + 1) * 128],
                )
                wsl_tiles[i] = t

            xg_map = {}

            def issue_xgrp(cc):
                tiles = []
                for g in range(KT // 2):
                    t = xgp.tile([128, 2 * CH], F32R, tag=f"xg{g}", name=f"xg{g}")
                    nc.sync.dma_start(
                        out=t[:].rearrange("p (kt s) -> p kt s", kt=2),
                        in_=xT3[:, g * 2:(g + 1) * 2, cc * CH:(cc + 1) * CH],
                    )
                    tiles.append(t)
                xg_map[cc] = tiles

            tab_map = {}

            def issue_tabs(cc):
                cs = slice(cc * CH, (cc + 1) * CH)
                cos_t = tabp.tile([128, CH], F32, tag="cos")
                nc.scalar.dma_start(out=cos_t[:], in_=cosT[:, cs])
                sin_t = tabp.tile([128, CH], F32, tag="sin")
                nc.scalar.dma_start(out=sin_t[:], in_=sinT[:, cs])
                tab_map[cc] = (cos_t, sin_t)

            qw_map = {}
            pend_rope = [None]

            def flush_rope():
                if pend_rope[0] is None:
                    return
                raw, dst, cos_t, sin_t = pend_rope[0]
                pend_rope[0] = None
                rotps = pshp.tile([128, CH], F32, tag="sh", name="rotps")
                nc.tensor.matmul(rotps[:], rT_sb[:], raw[:], start=True, stop=True)
                nc.vector.tensor_mul(dst, rotps[:], sin_t[:])
                t2 = t12p.tile([128, CH], F32, tag="t2", name="t2")
                nc.vector.tensor_mul(t2[:], raw[:].bitcast(F32), cos_t[:])
                nc.vector.tensor_add(dst, dst.bitcast(F32), t2[:])

            sw_idx = [0]

            def sub_wave():
                i = sw_idx[0]
                sw_idx[0] += 1
                qk, h, cc = SPECS[i]
                w_t = wsl_tiles.pop(i)
                xg = xg_map[cc]
                ps = ppp.tile([128, CH], F32, tag="proj", name="ps")
                for kt in range(KT):
                    xk_t = xg[kt // 2][:, (kt % 2) * CH:(kt % 2 + 1) * CH]
                    nc.tensor.matmul(
                        ps[:],
                        w_t[:, kt * 128:(kt + 1) * 128],
                        xk_t,
                        start=(kt == 0),
                        stop=(kt == KT - 1),
                    )
                issue_wsl(i + 2)
                flush_rope()
                raw = rawp.tile([128, CH], F32R, tag="raw", name="raw")
                nc.vector.tensor_copy(raw[:], ps[:])
                if qk == "k":
                    dst = krope[h][:, cc * CH:(cc + 1) * CH]
                else:
                    qt = qwp.tile([128, CH], F32R, tag=f"qw{h}", name=f"qw{h}")
                    qw_map.setdefault(cc, {})[h] = qt
                    dst = qt[:]
                cos_t, sin_t = tab_map[cc]
                pend_rope[0] = (raw, dst, cos_t, sin_t)

            # ---- attention pieces -------------------------------------------
            wv_sb = wvop.tile([128, KT * HG_], BF16, tag="wv")
            wo_sb = wvop.tile([128, NH_ * H_], BF16, tag="wo")

            def issue_wvo():
                for g in range(KT // 4):
                    nc.scalar.dma_start(
                        out=wv_sb[:, g * 4 * HG_:(g + 1) * 4 * HG_].rearrange(
                            "p (kt j) -> p kt j", kt=4
                        ),
                        in_=wv3[:, g * 4:(g + 1) * 4, :],
                    )
                nc.scalar.dma_start(
                    out=wo_sb[:].rearrange("p (j ho) -> p j ho", j=NH_),
                    in_=woT.rearrange("(j p) ho -> p j ho", p=128),
                )

            xv_map = {}

            def issue_xv(t):
                xv = xvp.tile([128, KT * 128], BF16, tag="xv")
                nc.scalar.dma_start(
                    out=xv[:].rearrange("p (kt s) -> p kt s", kt=KT),
                    in_=xbf3[:, :, t * 128:(t + 1) * 128],
                )
                xv_map[t] = xv

            def vproj_tile(t):
                xv = xv_map.pop(t)
                vps = pap.tile([128, HG_], F32, tag="big", name="vps")
                for kt in range(KT):
                    nc.tensor.matmul(
                        vps[:],
                        xv[:, kt * 128:(kt + 1) * 128],
                        wv_sb[:, kt * HG_:(kt + 1) * HG_],
                        start=(kt == 0),
                        stop=(kt == KT - 1),
                    )
                nc.scalar.copy(vslab[:, t * HG_:(t + 1) * HG_], vps[:])

            def do_transposes(ptg, pbf, sq, c):
                # 8 PE transposes share one 2KB PSUM bank (bf16), drained by a
                # single wide strided copy into the pt slab
                off = (sq - 4 * c) * 128
                ptv = ptg[:].rearrange("p (t ch) -> p t ch", ch=CH)
                ntr = sq + 1
                for g in range((ntr + 7) // 8):
                    gn = min(8, ntr - 8 * g)
                    bank = pshp.tile([128, 1024], BF16, tag="sh", name="trbank")
                    for u in range(gn):
                        nc.tensor.transpose(
                            bank[:, u * 128:(u + 1) * 128],
                            pbf[(8 * g + u) // 4][:, ((8 * g + u) % 4) * 128:
                                                  ((8 * g + u) % 4 + 1) * 128],
                            ident_sb[:],
                        )
                    src = bank[:].rearrange("p (t c) -> p t c", c=128)[:, :gn]
                    dst = ptv[:, 8 * g:8 * g + gn, off:off + 128]
                    if g % 2 == 1:
                        nc.scalar.copy(dst, src)
                    else:
                        nc.vector.tensor_copy(dst, src)

            def attn_scores(h, c):
                """Scores + softmax + P transposes for (h, c); last-sq
                transposes left pending (returned for finish_scores)."""
                rcp4 = statp.tile([128, 32], BF16, tag=f"rcp4_{h % 2}")
                ptg = ptp.tile([128, SQT * CH], BF16, tag="ptslab", name="ptslab")
                pend_tr = None
                for sq in range(4 * c, 4 * c + 4):
                    nch = sq // 4 + 1
                    ncols = (sq + 1) * 128
                    mx = statp.tile([128, NCHUNK], F32, tag="mx")
                    scps_list = []
                    for kc in range(nch):
                        cols = min(CH, ncols - kc * CH)
                        scps = pap.tile([128, CH], F32, tag="big", name="scps")
                        nc.tensor.matmul(
                            scps[:, :cols],
                            qw_map[c][h][:, (sq - 4 * c) * 128:(sq - 4 * c + 1) * 128],
                            krope[h][:, kc * CH: kc * CH + cols],
                            start=True,
                            stop=True,
                        )
                        if kc == nch - 1:
                            dcol = sq * 128 - kc * CH
                            nc.vector.tensor_add(
                                scps[:, dcol:dcol + 128],
                                scps[:, dcol:dcol + 128],
                                mask_sb[:],
                            )
                        if nch > 1:
                            nc.vector.tensor_reduce(
                                mx[:, kc:kc + 1], scps[:, :cols], axis=AX.X, op=ALU.max
                            )
                        scps_list.append((scps, cols))
                    negm = statp.tile([128, 1], F32, tag="negm")
                    if nch == 1:
                        scps0, cols0 = scps_list[0]
                        nc.vector.tensor_reduce(
                            negm[:], scps0[:, :cols0], axis=AX.X, op=ALU.max, negate=True
                        )
                    else:
                        nc.vector.tensor_reduce(
                            negm[:], mx[:, :nch], axis=AX.X, op=ALU.max, negate=True
                        )
                    pbf = [
                        pslabp.tile([128, CH], BF16, tag=f"pbf{kc}", name=f"pbf{kc}")
                        for kc in range(nch)
                    ]
                    ssum = statp.tile([128, NCHUNK], F32, tag="ssum")
                    for kc, (scps, cols) in enumerate(scps_list):
                        nc.scalar.activation(
                            pbf[kc][:, :cols],
                            scps[:, :cols],
                            ACTF.Exp,
                            bias=negm[:],
                            accum_out=ssum[:, kc:kc + 1],
                        )
                    rsum = statp.tile([128, 1], F32, tag="rsum")
                    nc.vector.tensor_reduce(
                        rsum[:], ssum[:, :nch], axis=AX.X, op=ALU.add
                    )
                    with nc.allow_low_precision(reason="bf16 softmax normalizer"):
                        nc.vector.reciprocal(rcp4[:, sq - 4 * c: sq - 4 * c + 1], rsum[:])
                    if pend_tr is not None:
                        do_transposes(ptg, *pend_tr, c)
                    pend_tr = (pbf, sq)
                return rcp4, ptg, pend_tr

            def finish_scores(h, c, state):
                rcp4, ptg, pend_tr = state
                do_transposes(ptg, *pend_tr, c)

            def attn_ctx(h, c, state):
                """P^T @ V and normalization for (h, c)."""
                rcp4, ptg, _ = state
                ptv = ptg[:].rearrange("p (t ch) -> p t ch", ch=CH)
                # reciprocal row broadcast: PE transposes rcp columns to a row,
                # GpSimd broadcasts it across partitions (no PE ones-matmul)
                bank = pshp.tile([128, 1024], BF16, tag="sh", name="rcpbank")
                for j in range(4):
                    nc.tensor.transpose(
                        bank[0:1, j * 128:(j + 1) * 128],
                        rcp4[:, j:j + 1],
                        ident_sb[:],
                    )
                rrow = rowp.tile([1, CH], BF16, tag="rrow")
                nc.scalar.copy(rrow[:], bank[0:1, 0:CH])
                bcsb = rowp.tile([128, CH], BF16, tag="bcsb")
                nc.gpsimd.partition_broadcast(bcsb[:], rrow[0:1, :])
                ctxps = pap.tile([128, CH], F32, tag="big", name="ctxps")
                tmax = 4 * c + 4
                for t in range(tmax):
                    c0 = max(0, (t - 4 * c) * 128)
                    nc.tensor.matmul(
                        ctxps[:, c0:CH],
                        vslab[:, t * HG_ + h * 128: t * HG_ + (h + 1) * 128],
                        ptv[:, t, c0:CH],
                        start=(t == 0),
                        stop=(t == tmax - 1),
                    )
                nc.vector.tensor_mul(
                    ctxT[h][:, (c % 2) * CH:(c % 2 + 1) * CH], ctxps[:], bcsb[:]
                )

            def out_proj_st(c, st):
                """Output projection for one 128-row tile st of chunk c."""
                ostg = ostp.tile([128, H_], BF16, tag="ostg", name="ostg")
                for hoc in range(H_ // CH):
                    wops = pap.tile([128, CH], F32, tag="big", name="wops")
                    for j in range(NH_):
                        nc.tensor.matmul(
                            wops[:],
                            ctxT[j][:, (c % 2) * CH + (st - 4 * c) * 128:
                                    (c % 2) * CH + (st - 4 * c + 1) * 128],
                            wo_sb[:, j * H_ + hoc * CH: j * H_ + (hoc + 1) * CH],
                            start=(j == 0),
                            stop=(j == NH_ - 1),
                        )
                    if hoc % 2 == 1:
                        nc.vector.tensor_copy(ostg[:, hoc * CH:(hoc + 1) * CH], wops[:])
                    else:
                        nc.scalar.copy(ostg[:, hoc * CH:(hoc + 1) * CH], wops[:])
                nc.sync.dma_start(out=out[st * 128:(st + 1) * 128, :], in_=ostg[:])

            # ---- prologue ----------------------------------------------------
            with nc.named_scope("prolog"):
                issue_wsl(0)
                issue_wsl(1)
                issue_tabs(0)
                issue_xgrp(0)
                for _ in range(8):
                    sub_wave()
                issue_wvo()

            # ---- main fused loop --------------------------------------------
            def scoped(nm, f, *a):
                with nc.named_scope(nm):
                    return f(*a)

            for c in range(NCHUNK):
                # fillers for this chunk: proj sub-waves for c+1, outproj c-1
                sws = [sub_wave] * (8 if c < NCHUNK - 1 else 0)
                ops = ([lambda st=st: out_proj_st(c - 1, st)
                        for st in range(4 * (c - 1), 4 * (c - 1) + 4)]
                       if c > 0 else [])
                if c < NCHUNK - 1:
                    fills = [
                        [sws[0]] if c < 3 else [],
                        ops[0:1],
                        sws[1:3],
                        ops[1:2],
                        sws[3:5],
                        ops[2:4] + sws[5:8],
                    ]
                else:
                    fills = [ops[0:1], ops[1:2], [], ops[2:3], [], ops[3:4]]

                def fill(i):
                    for f in fills[i]:
                        f()

                with nc.named_scope(f"c{c}"):
                    issue_xv(4 * c)
                    issue_xv(4 * c + 1)
                    if c < NCHUNK - 1:
                        issue_xgrp(c + 1)
                        issue_tabs(c + 1)
                    scoped(f"fr{c}", flush_rope)
                    s0 = scoped(f"s{c}h0", attn_scores, 0, c)
                    for t in range(4 * c, 4 * c + 4):
                        scoped(f"v{c}", vproj_tile, t)
                        if t + 2 < 4 * c + 4:
                            issue_xv(t + 2)
                    scoped(f"f{c}h0", finish_scores, 0, c, s0)
                    scoped(f"x{c}h0", attn_ctx, 0, c, s0)
                    s1 = scoped(f"s{c}h1", attn_scores, 1, c)
                    scoped(f"fl{c}a", fill, 0)
                    scoped(f"f{c}h1", finish_scores, 1, c, s1)
                    scoped(f"x{c}h1", attn_ctx, 1, c, s1)
                    scoped(f"fl{c}b", fill, 1)
                    s2 = scoped(f"s{c}h2", attn_scores, 2, c)
                    scoped(f"fl{c}c", fill, 2)
                    scoped(f"f{c}h2", finish_scores, 2, c, s2)
                    scoped(f"x{c}h2", attn_ctx, 2, c, s2)
                    scoped(f"fl{c}d", fill, 3)
                    s3 = scoped(f"s{c}h3", attn_scores, 3, c)
                    scoped(f"fl{c}e", fill, 4)
                    scoped(f"f{c}h3", finish_scores, 3, c, s3)
                    scoped(f"x{c}h3", attn_ctx, 3, c, s3)
                    scoped(f"fl{c}f", fill, 5)

            with nc.named_scope("epilog"):
                for st in range(4 * (NCHUNK - 1), 4 * NCHUNK):
                    out_proj_st(NCHUNK - 1, st)

    nc.compile()
    return nc


def _make_tables(S_, D_=128):
    inv_freq = 1.0 / (ROPE_BASE ** (np.arange(0, D_, 2, dtype=np.float32) / D_))
    pos = np.arange(S_, dtype=np.float32)
    ang = pos[:, None] * inv_freq[None, :]
    ang = np.concatenate([ang, ang], axis=1)
    return (
        np.cos(ang).T.astype(np.float32).copy(),
        np.sin(ang).T.astype(np.float32).copy(),
    )


def _make_rot_T(D_=128):
    R = np.zeros((D_, D_), dtype=np.float32)
    half = D_ // 2
    for d in range(half):
        R[d, d + half] = -1.0
    for d in range(half, D_):
        R[d, d - half] = 1.0
    return R.T.copy()


def _make_mask(mask_val=-1e30):
    m = np.zeros((128, 128), dtype=np.float32)
    m[np.triu_indices(128, k=1)] = mask_val
    return m


def kernel(x, Wq, Wk, Wv, Wo):
    """Full inputs in, full output out. Shards over 8 NeuronCores internally."""
    global LAST_RESULTS
    x = np.ascontiguousarray(np.asarray(x, dtype=np.float32))
    Wq = np.asarray(Wq, dtype=np.float32)
    Wk = np.asarray(Wk, dtype=np.float32)
    Wv = np.asarray(Wv, dtype=np.float32)
    Wo = np.asarray(Wo, dtype=np.float32)

    if "nc" not in _NC_CACHE:
        _NC_CACHE["nc"] = _build()
    nc = _NC_CACHE["nc"]

    scale = np.sqrt(np.float32(D))
    cosT, sinT = _make_tables(S)
    rT = _make_rot_T()
    identb = np.eye(128, dtype=ml_dtypes.bfloat16)
    maskt = _make_mask()

    WqT = Wq.T * scale                    # [H, 16*D], scale folded into q path
    WkT = np.ascontiguousarray(Wk.T)
    WvT_bf = Wv.T.astype(ml_dtypes.bfloat16)
    WoT_bf = Wo.T.astype(ml_dtypes.bfloat16)   # [H(in=ctx), H(out)]

    in_maps = []
    for c in range(N_CORES):
        b, g = divmod(c, NH)
        js = slice(g * HG, (g + 1) * HG)
        xT_b = np.ascontiguousarray(x[b].T)
        in_maps.append({
            "xT": xT_b,
            "xbfT": xT_b.astype(ml_dtypes.bfloat16),
            "wqT": np.ascontiguousarray(WqT[:, js]).astype(np.float32),
            "wkT": np.ascontiguousarray(WkT[:, js]),
            "wvT": np.ascontiguousarray(WvT_bf[:, js]),
            "woT": np.ascontiguousarray(WoT_bf[js, :]),
            "cosT": cosT,
            "sinT": sinT,
            "rT": rT,
            "ident": identb,
            "mask": maskt,
        })

    LAST_RESULTS = run_bass_kernel_spmd(
        nc, in_maps, core_ids=list(range(N_CORES)), trace=TRACE
    )
    res = LAST_RESULTS.results

    outv = np.zeros((B, S, H), dtype=np.float32)
    for c in range(N_CORES):
        b = c // NH
        outv[b] += res[c]["out"].astype(np.float32)
    return outv


# revision 18
# speedup vs baseline: 1.0537x; 1.0537x over previous
"""Self-contained Trainium2 kernel for nn_AutoregressiveGroupQuerySelfAttention.

Reference computation (B=2, S=2048, H=2048, 16 heads x 128 dim):
    q = (x @ Wq.T) -> heads; k likewise; v likewise
    q, k get RoPE; scores = (q @ k.T) * sqrt(D)   (faithful-to-source bug)
    causal softmax; ctx = attn @ v; out = ctx @ Wo.T

Sharding over 8 NeuronCores: core c = (b, g) with b = c // 4 (batch),
g = c % 4 (head-group of 4 heads = 512 hidden columns).  Each core computes
its head-group's context and a partial output  ctx_g @ Wo.T[g-rows, :];
the host sums the 4 partials per batch element.

Precision: logit path (q/k projections, rope, scores) in fp32r; value path
(v, P, Wo) in bf16; output stored bf16 and accumulated f32 on host.

Schedule: fully fused single phase.  The q/k projection for chunk c+1 is
split into 8 per-head "sub-waves" (16 accumulating matmuls each) that are
interleaved into chunk c's attention as PE filler, so the Tensor engine
never idles long enough for the HAM clock-gate to downclock it.  Weight
slices for the sub-waves are re-streamed from HBM each chunk (SBUF is too
small to hold them and all activation slabs), prefetched 3 sub-waves ahead.
The softmax reciprocal is broadcast across partitions via GpSimd instead of
a PE ones-matmul.  PSUM: 2 banks proj accumulators, 2 shared rope/transpose
banks, 4 attention banks.
"""
import numpy as np
import ml_dtypes

import concourse.bass as bass
import concourse.mybir as mybir
from concourse import bacc
from concourse.tile import TileContext
from concourse.bass_utils import run_bass_kernel_spmd

F32 = mybir.dt.float32
F32R = mybir.dt.float32r
BF16 = mybir.dt.bfloat16
AX = mybir.AxisListType
ALU = mybir.AluOpType
ACTF = mybir.ActivationFunctionType

B, S, H = 2, 2048, 2048
NUM_HEADS, D = 16, 128
N_CORES = 8
NH = 4                     # heads per core
HG = NH * D                # 512
ROPE_BASE = 10000.0

_NC_CACHE = {}
LAST_RESULTS = None        # BassKernelResults of the most recent run (for profiling)
TRACE = False


def _build(S_=S, H_=H, NH_=NH):
    DD = 128
    HG_ = NH_ * DD
    KT = H_ // 128
    SQT = S_ // 128
    CH = 512
    NCHUNK = S_ // CH

    nc = bacc.Bacc()
    xT = nc.declare_dram_parameter("xT", [H_, S_], F32R, isOutput=False)
    # xbfS[t*128+p, kt*128+s] = x[t*128+s, kt*128+p] — per-vproj-tile slices
    # are fully contiguous per partition (4KB lines)
    xbfS = nc.declare_dram_parameter("xbfS", [SQT * 128, KT * 128], BF16,
                                     isOutput=False)
    # wqS[h*128+p, kt*128+j] = Wq^T[kt*128+p, h*128+j] (scale folded) — the
    # per-head sub-wave weight slice is contiguous per partition (8KB lines)
    wqS = nc.declare_dram_parameter("wqS", [NH_ * 128, KT * 128], F32R,
                                    isOutput=False)
    wkS = nc.declare_dram_parameter("wkS", [NH_ * 128, KT * 128], F32R,
                                    isOutput=False)
    wvT = nc.declare_dram_parameter("wvT", [H_, HG_], BF16, isOutput=False)
    woT = nc.declare_dram_parameter("woT", [HG_, H_], BF16, isOutput=False)
    cosT = nc.declare_dram_parameter("cosT", [128, S_], F32, isOutput=False)
    sinT = nc.declare_dram_parameter("sinT", [128, S_], F32, isOutput=False)
    rT = nc.declare_dram_parameter("rT", [128, 128], F32R, isOutput=False)
    ident = nc.declare_dram_parameter("ident", [128, 128], BF16, isOutput=False)
    onesr = nc.declare_dram_parameter("onesr", [1, 128], BF16, isOutput=False)
    mask = nc.declare_dram_parameter("mask", [128, 128], F32, isOutput=False)
    out = nc.declare_dram_parameter("out", [S_, H_], BF16, isOutput=True)

    wv3 = wvT.rearrange("(kt p) j -> p kt j", p=128)
    xT3 = xT.rearrange("(kt p) s -> p kt s", p=128)

    from contextlib import ExitStack
    with TileContext(nc) as tc:
        with ExitStack() as _stk:
            def _pool(**kw):
                return _stk.enter_context(tc.tile_pool(**kw))
            slabp = _pool(name="slabs", bufs=1)
            qwp = _pool(name="qw", bufs=2)
            wstp = _pool(name="wst", bufs=2)
            wvop = _pool(name="wvo", bufs=1)
            xgp = _pool(name="xg", bufs=1)
            xvp = _pool(name="xv", bufs=2)
            tabp = _pool(name="tab", bufs=1)
            rawp = _pool(name="raw", bufs=2)
            t12p = _pool(name="t12", bufs=1)
            pslabp = _pool(name="pslab", bufs=2)
            ptp = _pool(name="ptpool", bufs=1)
            ctxp = _pool(name="ctxpool", bufs=1)
            ostp = _pool(name="ostage", bufs=2)
            statp = _pool(name="stats", bufs=3)
            rowp = _pool(name="rows", bufs=1)
            ppp = _pool(name="pp", bufs=2, space="PSUM")
            pshp = _pool(name="psh", bufs=2, space="PSUM")
            pap = _pool(name="pa", bufs=4, space="PSUM")
            # ---- persistent slabs
            krope = [slabp.tile([128, S_], F32R, tag=f"krope{h}", name=f"krope{h}")
                     for h in range(NH_)]
            vslab = slabp.tile([128, SQT * HG_], BF16, tag="vslab")
            ident_sb = slabp.tile([128, 128], BF16, tag="ident")
            nc.scalar.dma_start(out=ident_sb[:], in_=ident[:])
            mask_sb = slabp.tile([128, 128], F32, tag="mask")
            nc.scalar.dma_start(out=mask_sb[:], in_=mask[:])
            ones_sb = slabp.tile([1, 128], BF16, tag="onesr")
            nc.scalar.dma_start(out=ones_sb[:], in_=onesr[:])
            rT_sb = slabp.tile([128, 128], F32R, tag="rT")
            nc.scalar.dma_start(out=rT_sb[:], in_=rT[:])

            ctxT = [ctxp.tile([128, 2 * CH], BF16, tag=f"ctxT{h}", name=f"ctxT{h}")
                    for h in range(NH_)]

            # ---- proj sub-wave machinery ------------------------------------
            # SPECS: (qk, h, cc) — sub-wave computing q/k head h of chunk cc.
            SPECS = [(qk, h, cc) for cc in range(NCHUNK)
                     for qk in ("k", "q") for h in range(NH_)]
            wsl_tiles = {}

            def issue_wsl(i):
                if i >= len(SPECS):
                    return
                qk, h, cc = SPECS[i]
                t = wstp.tile([128, KT * 128], F32R, tag="wsl", name=f"w{qk}{h}c{cc}")
                wS = wqS if qk == "q" else wkS
                nc.scalar.dma_start(
                    out=t[:], in_=wS[h * 128:(h + 1) * 128, :]
                )
                wsl_tiles[i] = t

            xg_map = {}

            def issue_xgrp(cc):
                tiles = []
                for g in range(KT // 2):
                    t = xgp.tile([128, 2 * CH], F32R, tag=f"xg{g}", name=f"xg{g}")
                    nc.sync.dma_start(
                        out=t[:].rearrange("p (kt s) -> p kt s", kt=2),
                        in_=xT3[:, g * 2:(g + 1) * 2, cc * CH:(cc + 1) * CH],
                    )
                    tiles.append(t)
                xg_map[cc] = tiles

            tab_map = {}

            def issue_tabs(cc):
                cs = slice(cc * CH, (cc + 1) * CH)
                cos_t = tabp.tile([128, CH], F32, tag="cos")
                nc.scalar.dma_start(out=cos_t[:], in_=cosT[:, cs])
                sin_t = tabp.tile([128, CH], F32, tag="sin")
                nc.scalar.dma_start(out=sin_t[:], in_=sinT[:, cs])
                tab_map[cc] = (cos_t, sin_t)

            qw_map = {}
            pend_rope = [None]

            def flush_rope():
                if pend_rope[0] is None:
                    return
                raw, dst, cos_t, sin_t = pend_rope[0]
                pend_rope[0] = None
                rotps = pshp.tile([128, CH], F32, tag="sh", name="rotps")
                nc.tensor.matmul(rotps[:], rT_sb[:], raw[:], start=True, stop=True)
                nc.vector.tensor_mul(dst, rotps[:], sin_t[:])
                t2 = t12p.tile([128, CH], F32, tag="t2", name="t2")
                nc.vector.tensor_mul(t2[:], raw[:].bitcast(F32), cos_t[:])
                nc.vector.tensor_add(dst, dst.bitcast(F32), t2[:])

            sw_idx = [0]

            def sub_wave():
                i = sw_idx[0]
                sw_idx[0] += 1
                qk, h, cc = SPECS[i]
                w_t = wsl_tiles.pop(i)
                xg = xg_map[cc]
                ps = ppp.tile([128, CH], F32, tag="proj", name="ps")
                for kt in range(KT):
                    xk_t = xg[kt // 2][:, (kt % 2) * CH:(kt % 2 + 1) * CH]
                    nc.tensor.matmul(
                        ps[:],
                        w_t[:, kt * 128:(kt + 1) * 128],
                        xk_t,
                        start=(kt == 0),
                        stop=(kt == KT - 1),
                    )
                issue_wsl(i + 2)
                flush_rope()
                raw = rawp.tile([128, CH], F32R, tag="raw", name="raw")
                nc.vector.tensor_copy(raw[:], ps[:])
                if qk == "k":
                    dst = krope[h][:, cc * CH:(cc + 1) * CH]
                else:
                    qt = qwp.tile([128, CH], F32R, tag=f"qw{h}", name=f"qw{h}")
                    qw_map.setdefault(cc, {})[h] = qt
                    dst = qt[:]
                cos_t, sin_t = tab_map[cc]
                pend_rope[0] = (raw, dst, cos_t, sin_t)

            # ---- attention pieces -------------------------------------------
            wv_sb = wvop.tile([128, KT * HG_], BF16, tag="wv")
            wo_sb = wvop.tile([128, NH_ * H_], BF16, tag="wo")

            def issue_wvo():
                for g in range(KT // 4):
                    nc.scalar.dma_start(
                        out=wv_sb[:, g * 4 * HG_:(g + 1) * 4 * HG_].rearrange(
                            "p (kt j) -> p kt j", kt=4
                        ),
                        in_=wv3[:, g * 4:(g + 1) * 4, :],
                    )
                nc.scalar.dma_start(
                    out=wo_sb[:].rearrange("p (j ho) -> p j ho", j=NH_),
                    in_=woT.rearrange("(j p) ho -> p j ho", p=128),
                )

            xv_map = {}

            def issue_xv(t):
                xv = xvp.tile([128, KT * 128], BF16, tag="xv")
                nc.scalar.dma_start(
                    out=xv[:], in_=xbfS[t * 128:(t + 1) * 128, :]
                )
                xv_map[t] = xv

            def vproj_tile(t):
                xv = xv_map.pop(t)
                vps = pap.tile([128, HG_], F32, tag="big", name="vps")
                for kt in range(KT):
                    nc.tensor.matmul(
                        vps[:],
                        xv[:, kt * 128:(kt + 1) * 128],
                        wv_sb[:, kt * HG_:(kt + 1) * HG_],
                        start=(kt == 0),
                        stop=(kt == KT - 1),
                    )
                nc.scalar.copy(vslab[:, t * HG_:(t + 1) * HG_], vps[:])

            def do_transposes(ptg, pbf, sq, c):
                # 8 PE transposes share one 2KB PSUM bank (bf16), drained by a
                # single wide strided copy into the pt slab
                off = (sq - 4 * c) * 128
                ptv = ptg[:].rearrange("p (t ch) -> p t ch", ch=CH)
                ntr = sq + 1
                for g in range((ntr + 7) // 8):
                    gn = min(8, ntr - 8 * g)
                    bank = pshp.tile([128, 1024], BF16, tag="sh", name="trbank")
                    for u in range(gn):
                        nc.tensor.transpose(
                            bank[:, u * 128:(u + 1) * 128],
                            pbf[(8 * g + u) // 4][:, ((8 * g + u) % 4) * 128:
                                                  ((8 * g + u) % 4 + 1) * 128],
                            ident_sb[:],
                        )
                    src = bank[:].rearrange("p (t c) -> p t c", c=128)[:, :gn]
                    dst = ptv[:, 8 * g:8 * g + gn, off:off + 128]
                    if g % 2 == 1:
                        nc.scalar.copy(dst, src)
                    else:
                        nc.vector.tensor_copy(dst, src)

            def attn_scores(h, c):
                """Scores + softmax + P transposes for (h, c); last-sq
                transposes left pending (returned for finish_scores)."""
                rcp4 = statp.tile([128, 32], BF16, tag=f"rcp4_{h % 2}")
                ptg = ptp.tile([128, SQT * CH], BF16, tag="ptslab", name="ptslab")
                pend_tr = None
                for sq in range(4 * c, 4 * c + 4):
                    nch = sq // 4 + 1
                    ncols = (sq + 1) * 128
                    mx = statp.tile([128, NCHUNK], F32, tag="mx")
                    scps_list = []
                    for kc in range(nch):
                        cols = min(CH, ncols - kc * CH)
                        scps = pap.tile([128, CH], F32, tag="big", name="scps")
                        nc.tensor.matmul(
                            scps[:, :cols],
                            qw_map[c][h][:, (sq - 4 * c) * 128:(sq - 4 * c + 1) * 128],
                            krope[h][:, kc * CH: kc * CH + cols],
                            start=True,
                            stop=True,
                        )
                        if kc == nch - 1:
                            dcol = sq * 128 - kc * CH
                            nc.vector.tensor_add(
                                scps[:, dcol:dcol + 128],
                                scps[:, dcol:dcol + 128],
                                mask_sb[:],
                            )
                        if nch > 1:
                            nc.vector.tensor_reduce(
                                mx[:, kc:kc + 1], scps[:, :cols], axis=AX.X, op=ALU.max
                            )
                        scps_list.append((scps, cols))
                    negm = statp.tile([128, 1], F32, tag="negm")
                    if nch == 1:
                        scps0, cols0 = scps_list[0]
                        nc.vector.tensor_reduce(
                            negm[:], scps0[:, :cols0], axis=AX.X, op=ALU.max, negate=True
                        )
                    else:
                        nc.vector.tensor_reduce(
                            negm[:], mx[:, :nch], axis=AX.X, op=ALU.max, negate=True
                        )
                    pbf = [
                        pslabp.tile([128, CH], BF16, tag=f"pbf{kc}", name=f"pbf{kc}")
                        for kc in range(nch)
                    ]
                    ssum = statp.tile([128, NCHUNK], F32, tag="ssum")
                    for kc, (scps, cols) in enumerate(scps_list):
                        nc.scalar.activation(
                            pbf[kc][:, :cols],
                            scps[:, :cols],
                            ACTF.Exp,
                            bias=negm[:],
                            accum_out=ssum[:, kc:kc + 1],
                        )
                    rsum = statp.tile([128, 1], F32, tag="rsum")
                    nc.vector.tensor_reduce(
                        rsum[:], ssum[:, :nch], axis=AX.X, op=ALU.add
                    )
                    with nc.allow_low_precision(reason="bf16 softmax normalizer"):
                        nc.vector.reciprocal(rcp4[:, sq - 4 * c: sq - 4 * c + 1], rsum[:])
                    if pend_tr is not None:
                        do_transposes(ptg, *pend_tr, c)
                    pend_tr = (pbf, sq)
                return rcp4, ptg, pend_tr

            def finish_scores(h, c, state):
                rcp4, ptg, pend_tr = state
                do_transposes(ptg, *pend_tr, c)

            def attn_ctx(h, c, state):
                """P^T @ V and normalization for (h, c)."""
                rcp4, ptg, _ = state
                ptv = ptg[:].rearrange("p (t ch) -> p t ch", ch=CH)
                # reciprocal row broadcast: PE transposes rcp columns to a row,
                # GpSimd broadcasts it across partitions (no PE ones-matmul)
                bank = pshp.tile([128, 1024], BF16, tag="sh", name="rcpbank")
                for j in range(4):
                    nc.tensor.transpose(
                        bank[0:1, j * 128:(j + 1) * 128],
                        rcp4[:, j:j + 1],
                        ident_sb[:],
                    )
                rrow = rowp.tile([1, CH], BF16, tag="rrow")
                nc.scalar.copy(rrow[:], bank[0:1, 0:CH])
                bcps = pshp.tile([128, CH], F32, tag="sh", name="bcps")
                nc.tensor.matmul(bcps[:], ones_sb[:], rrow[:], start=True, stop=True)
                bcsb = rowp.tile([128, CH], BF16, tag="bcsb")
                nc.scalar.copy(bcsb[:], bcps[:])
                ctxps = pap.tile([128, CH], F32, tag="big", name="ctxps")
                tmax = 4 * c + 4
                for t in range(tmax):
                    c0 = max(0, (t - 4 * c) * 128)
                    nc.tensor.matmul(
                        ctxps[:, c0:CH],
                        vslab[:, t * HG_ + h * 128: t * HG_ + (h + 1) * 128],
                        ptv[:, t, c0:CH],
                        start=(t == 0),
                        stop=(t == tmax - 1),
                    )
                nc.vector.tensor_mul(
                    ctxT[h][:, (c % 2) * CH:(c % 2 + 1) * CH], ctxps[:], bcsb[:]
                )

            def out_proj_st(c, st):
                """Output projection for one 128-row tile st of chunk c."""
                ostg = ostp.tile([128, H_], BF16, tag="ostg", name="ostg")
                for hoc in range(H_ // CH):
                    wops = pap.tile([128, CH], F32, tag="big", name="wops")
                    for j in range(NH_):
                        nc.tensor.matmul(
                            wops[:],
                            ctxT[j][:, (c % 2) * CH + (st - 4 * c) * 128:
                                    (c % 2) * CH + (st - 4 * c + 1) * 128],
                            wo_sb[:, j * H_ + hoc * CH: j * H_ + (hoc + 1) * CH],
                            start=(j == 0),
                            stop=(j == NH_ - 1),
                        )
                    if hoc % 2 == 1:
                        nc.vector.tensor_copy(ostg[:, hoc * CH:(hoc + 1) * CH], wops[:])
                    else:
                        nc.scalar.copy(ostg[:, hoc * CH:(hoc + 1) * CH], wops[:])
                nc.sync.dma_start(out=out[st * 128:(st + 1) * 128, :], in_=ostg[:])

            # ---- prologue ----------------------------------------------------
            with nc.named_scope("prolog"):
                issue_wsl(0)
                issue_wsl(1)
                issue_tabs(0)
                issue_xgrp(0)
                for _ in range(8):
                    sub_wave()
                issue_wvo()

            # ---- main fused loop --------------------------------------------
            def scoped(nm, f, *a):
                with nc.named_scope(nm):
                    return f(*a)

            for c in range(NCHUNK):
                # fillers for this chunk: proj sub-waves for c+1, outproj c-1
                sws = [sub_wave] * (8 if c < NCHUNK - 1 else 0)
                ops = ([lambda st=st: out_proj_st(c - 1, st)
                        for st in range(4 * (c - 1), 4 * (c - 1) + 4)]
                       if c > 0 else [])
                if c < NCHUNK - 1:
                    fills = [
                        [sws[0]] if c < 3 else [],
                        ops[0:1],
                        sws[1:3],
                        ops[1:2],
                        sws[3:5],
                        ops[2:4] + sws[5:8],
                    ]
                else:
                    fills = [ops[0:1], ops[1:2], [], ops[2:3], [], ops[3:4]]

                def fill(i):
                    for f in fills[i]:
                        f()

                with nc.named_scope(f"c{c}"):
                    issue_xv(4 * c)
                    issue_xv(4 * c + 1)
                    if c < NCHUNK - 1:
                        issue_xgrp(c + 1)
                        issue_tabs(c + 1)
                    scoped(f"fr{c}", flush_rope)
                    s0 = scoped(f"s{c}h0", attn_scores, 0, c)
                    for t in range(4 * c, 4 * c + 4):
                        scoped(f"v{c}", vproj_tile, t)
                        if t + 2 < 4 * c + 4:
                            issue_xv(t + 2)
                    scoped(f"f{c}h0", finish_scores, 0, c, s0)
                    scoped(f"x{c}h0", attn_ctx, 0, c, s0)
                    s1 = scoped(f"s{c}h1", attn_scores, 1, c)
                    scoped(f"fl{c}a", fill, 0)
                    scoped(f"f{c}h1", finish_scores, 1, c, s1)
                    scoped(f"x{c}h1", attn_ctx, 1, c, s1)
                    scoped(f"fl{c}b", fill, 1)
                    s2 = scoped(f"s{c}h2", attn_scores, 2, c)
                    scoped(f"fl{c}c", fill, 2)
                    scoped(f"f{c}h2", finish_scores, 2, c, s2)
                    scoped(f"x{c}h2", attn_ctx, 2, c, s2)
                    scoped(f"fl{c}d", fill, 3)
                    s3 = scoped(f"s{c}h3", attn_scores, 3, c)
                    scoped(f"fl{c}e", fill, 4)
                    scoped(f"f{c}h3", finish_scores, 3, c, s3)
                    scoped(f"x{c}h3", attn_ctx, 3, c, s3)
                    scoped(f"fl{c}f", fill, 5)

            with nc.named_scope("epilog"):
                for st in range(4 * (NCHUNK - 1), 4 * NCHUNK):
                    out_proj_st(NCHUNK - 1, st)

    nc.compile()
    return nc


def _make_tables(S_, D_=128):
    inv_freq = 1.0 / (ROPE_BASE ** (np.arange(0, D_, 2, dtype=np.float32) / D_))
    pos = np.arange(S_, dtype=np.float32)
    ang = pos[:, None] * inv_freq[None, :]
    ang = np.concatenate([ang, ang], axis=1)
    return (
        np.cos(ang).T.astype(np.float32).copy(),
        np.sin(ang).T.astype(np.float32).copy(),
    )


def _make_rot_T(D_=128):
    R = np.zeros((D_, D_), dtype=np.float32)
    half = D_ // 2
    for d in range(half):
        R[d, d + half] = -1.0
    for d in range(half, D_):
        R[d, d - half] = 1.0
    return R.T.copy()


def _make_mask(mask_val=-1e30):
    m = np.zeros((128, 128), dtype=np.float32)
    m[np.triu_indices(128, k=1)] = mask_val
    return m


def kernel(x, Wq, Wk, Wv, Wo):
    """Full inputs in, full output out. Shards over 8 NeuronCores internally."""
    global LAST_RESULTS
    x = np.ascontiguousarray(np.asarray(x, dtype=np.float32))
    Wq = np.asarray(Wq, dtype=np.float32)
    Wk = np.asarray(Wk, dtype=np.float32)
    Wv = np.asarray(Wv, dtype=np.float32)
    Wo = np.asarray(Wo, dtype=np.float32)

    if "nc" not in _NC_CACHE:
        _NC_CACHE["nc"] = _build()
    nc = _NC_CACHE["nc"]

    scale = np.sqrt(np.float32(D))
    cosT, sinT = _make_tables(S)
    rT = _make_rot_T()
    identb = np.eye(128, dtype=ml_dtypes.bfloat16)
    maskt = _make_mask()

    WqT = Wq.T * scale                    # [H, 16*D], scale folded into q path
    WkT = np.ascontiguousarray(Wk.T)
    WvT_bf = Wv.T.astype(ml_dtypes.bfloat16)
    WoT_bf = Wo.T.astype(ml_dtypes.bfloat16)   # [H(in=ctx), H(out)]

    KT, SQT = H // 128, S // 128

    def _pack_w(Wsl):
        # [H, HG] -> [NH*128, KT*128]: row h*128+p, col kt*128+j
        return np.ascontiguousarray(
            Wsl.reshape(KT, 128, NH, 128).transpose(2, 1, 0, 3).reshape(
                NH * 128, KT * 128)
        )

    in_maps = []
    for c in range(N_CORES):
        b, g = divmod(c, NH)
        js = slice(g * HG, (g + 1) * HG)
        xT_b = np.ascontiguousarray(x[b].T)
        xbfS = np.ascontiguousarray(
            xT_b.astype(ml_dtypes.bfloat16).reshape(KT, 128, SQT, 128)
            .transpose(2, 1, 0, 3).reshape(SQT * 128, KT * 128)
        )
        in_maps.append({
            "xT": xT_b,
            "xbfS": xbfS,
            "wqS": _pack_w(WqT[:, js]).astype(np.float32),
            "wkS": _pack_w(WkT[:, js]),
            "wvT": np.ascontiguousarray(WvT_bf[:, js]),
            "woT": np.ascontiguousarray(WoT_bf[js, :]),
            "cosT": cosT,
            "sinT": sinT,
            "rT": rT,
            "ident": identb,
            "onesr": np.ones((1, 128), dtype=ml_dtypes.bfloat16),
            "mask": maskt,
        })

    LAST_RESULTS = run_bass_kernel_spmd(
        nc, in_maps, core_ids=list(range(N_CORES)), trace=TRACE
    )
    res = LAST_RESULTS.results

    outv = np.zeros((B, S, H), dtype=np.float32)
    for c in range(N_CORES):
        b = c // NH
        outv[b] += res[c]["out"].astype(np.float32)
    return outv


# revision 23
# speedup vs baseline: 1.1673x; 1.1079x over previous
"""Self-contained Trainium2 kernel for nn_AutoregressiveGroupQuerySelfAttention.

Reference computation (B=2, S=2048, H=2048, 16 heads x 128 dim):
    q = (x @ Wq.T) -> heads; k likewise; v likewise
    q, k get RoPE; scores = (q @ k.T) * sqrt(D)   (faithful-to-source bug)
    causal softmax; ctx = attn @ v; out = ctx @ Wo.T

Sharding over 8 NeuronCores: core c = (b, g) with b = c // 4 (batch),
g = c % 4 (head-group of 4 heads = 512 hidden columns).  Each core computes
its head-group's context and a partial output  ctx_g @ Wo.T[g-rows, :];
the host sums the 4 partials per batch element.

Precision: logit path (q/k projections, rope, scores) in fp32r; value path
(v, P, Wo) in bf16; output stored bf16 and accumulated f32 on host.

Schedule: fully fused single phase.  The q/k projection for chunk c+1 is
split into 8 per-head "sub-waves" (16 accumulating matmuls each) that are
interleaved into chunk c's attention as PE filler, so the Tensor engine
never idles long enough for the HAM clock-gate to downclock it.  Weight
slices for the sub-waves are re-streamed from HBM each chunk (SBUF is too
small to hold them and all activation slabs), prefetched 3 sub-waves ahead.
The softmax reciprocal is broadcast across partitions via GpSimd instead of
a PE ones-matmul.  PSUM: 2 banks proj accumulators, 2 shared rope/transpose
banks, 4 attention banks.
"""
import numpy as np
import ml_dtypes

import concourse.bass as bass
import concourse.mybir as mybir
from concourse import bacc
from concourse.tile import TileContext
from concourse.bass_utils import run_bass_kernel_spmd

F32 = mybir.dt.float32
F32R = mybir.dt.float32r
BF16 = mybir.dt.bfloat16
AX = mybir.AxisListType
ALU = mybir.AluOpType
ACTF = mybir.ActivationFunctionType

B, S, H = 2, 2048, 2048
NUM_HEADS, D = 16, 128
N_CORES = 8
NH = 4                     # heads per core
HG = NH * D                # 512
ROPE_BASE = 10000.0

_NC_CACHE = {}
LAST_RESULTS = None        # BassKernelResults of the most recent run (for profiling)
TRACE = False


def _build(S_=S, H_=H, NH_=NH):
    DD = 128
    HG_ = NH_ * DD
    KT = H_ // 128
    SQT = S_ // 128
    CH = 512
    NCHUNK = S_ // CH

    nc = bacc.Bacc()
    xT = nc.declare_dram_parameter("xT", [H_, S_], F32R, isOutput=False)
    # xbfS[t*128+p, kt*128+s] = x[t*128+s, kt*128+p] — per-vproj-tile slices
    # are fully contiguous per partition (4KB lines)
    xbfS = nc.declare_dram_parameter("xbfS", [SQT * 128, KT * 128], BF16,
                                     isOutput=False)
    # wqS[h*128+p, kt*128+j] = Wq^T[kt*128+p, h*128+j] (scale folded) — the
    # per-head sub-wave weight slice is contiguous per partition (8KB lines)
    wqS = nc.declare_dram_parameter("wqS", [NH_ * 128, KT * 128], F32R,
                                    isOutput=False)
    wkS = nc.declare_dram_parameter("wkS", [NH_ * 128, KT * 128], F32R,
                                    isOutput=False)
    wvT = nc.declare_dram_parameter("wvT", [H_, HG_], BF16, isOutput=False)
    woT = nc.declare_dram_parameter("woT", [HG_, H_], BF16, isOutput=False)
    cosT = nc.declare_dram_parameter("cosT", [128, S_], F32, isOutput=False)
    sinT = nc.declare_dram_parameter("sinT", [128, S_], F32, isOutput=False)
    rT = nc.declare_dram_parameter("rT", [128, 128], F32R, isOutput=False)
    ident = nc.declare_dram_parameter("ident", [128, 128], BF16, isOutput=False)
    onesr = nc.declare_dram_parameter("onesr", [1, 128], BF16, isOutput=False)
    mask = nc.declare_dram_parameter("mask", [128, 128], F32, isOutput=False)
    out = nc.declare_dram_parameter("out", [S_, H_], BF16, isOutput=True)

    wv3 = wvT.rearrange("(kt p) j -> p kt j", p=128)
    xT3 = xT.rearrange("(kt p) s -> p kt s", p=128)

    from contextlib import ExitStack
    with TileContext(nc) as tc:
        with ExitStack() as _stk:
            def _pool(**kw):
                return _stk.enter_context(tc.tile_pool(**kw))
            slabp = _pool(name="slabs", bufs=1)
            qwp = _pool(name="qw", bufs=2)
            wstp = _pool(name="wst", bufs=2)
            wvop = _pool(name="wvo", bufs=1)
            xgp = _pool(name="xg", bufs=1)
            xvp = _pool(name="xv", bufs=2)
            tabp = _pool(name="tab", bufs=1)
            rawp = _pool(name="raw", bufs=2)
            t12p = _pool(name="t12", bufs=1)
            pslabp = _pool(name="pslab", bufs=2)
            ptp = _pool(name="ptpool", bufs=1)
            ctxp = _pool(name="ctxpool", bufs=1)
            ostp = _pool(name="ostage", bufs=2)
            statp = _pool(name="stats", bufs=3)
            rowp = _pool(name="rows", bufs=1)
            pshp = _pool(name="psh", bufs=2, space="PSUM")
            pap = _pool(name="pa", bufs=2, space="PSUM")
            pscp = _pool(name="psc", bufs=2, space="PSUM")
            # ---- persistent slabs
            krope = [slabp.tile([128, S_], F32R, tag=f"krope{h}", name=f"krope{h}")
                     for h in range(NH_)]
            vslab = slabp.tile([128, SQT * HG_], BF16, tag="vslab")
            ident_sb = slabp.tile([128, 128], BF16, tag="ident")
            nc.scalar.dma_start(out=ident_sb[:], in_=ident[:])
            mask_sb = slabp.tile([128, 128], F32, tag="mask")
            nc.scalar.dma_start(out=mask_sb[:], in_=mask[:])
            ones_sb = slabp.tile([1, 128], BF16, tag="onesr")
            nc.scalar.dma_start(out=ones_sb[:], in_=onesr[:])
            rT_sb = slabp.tile([128, 128], F32R, tag="rT")
            nc.scalar.dma_start(out=rT_sb[:], in_=rT[:])

            ctxT = [ctxp.tile([128, 2 * CH], BF16, tag=f"ctxT{h}", name=f"ctxT{h}")
                    for h in range(NH_)]

            # ---- proj sub-wave machinery ------------------------------------
            # SPECS: (qk, h, cc) — sub-wave computing q/k head h of chunk cc.
            SPECS = [(qk, h, cc) for cc in range(NCHUNK)
                     for qk in ("k", "q") for h in range(NH_)]
            wsl_tiles = {}

            def issue_wsl(i):
                if i >= len(SPECS):
                    return
                qk, h, cc = SPECS[i]
                t = wstp.tile([128, KT * 128], F32R, tag="wsl", name=f"w{qk}{h}c{cc}")
                wS = wqS if qk == "q" else wkS
                nc.scalar.dma_start(
                    out=t[:], in_=wS[h * 128:(h + 1) * 128, :]
                )
                wsl_tiles[i] = t

            xg_map = {}

            def issue_xgrp(cc):
                tiles = []
                for g in range(KT // 2):
                    t = xgp.tile([128, 2 * CH], F32R, tag=f"xg{g}", name=f"xg{g}")
                    nc.sync.dma_start(
                        out=t[:].rearrange("p (kt s) -> p kt s", kt=2),
                        in_=xT3[:, g * 2:(g + 1) * 2, cc * CH:(cc + 1) * CH],
                    )
                    tiles.append(t)
                xg_map[cc] = tiles

            tab_map = {}

            def issue_tabs(cc):
                cs = slice(cc * CH, (cc + 1) * CH)
                cos_t = tabp.tile([128, CH], F32, tag="cos")
                nc.scalar.dma_start(out=cos_t[:], in_=cosT[:, cs])
                sin_t = tabp.tile([128, CH], F32, tag="sin")
                nc.scalar.dma_start(out=sin_t[:], in_=sinT[:, cs])
                tab_map[cc] = (cos_t, sin_t)

            qw_map = {}
            pend_rope = [None]

            def flush_rope():
                if pend_rope[0] is None:
                    return
                raw, dst, cos_t, sin_t = pend_rope[0]
                pend_rope[0] = None
                rotps = pshp.tile([128, CH], F32, tag="sh", name="rotps")
                nc.tensor.matmul(rotps[:], rT_sb[:], raw[:], start=True, stop=True)
                nc.vector.tensor_mul(dst, rotps[:], sin_t[:])
                t2 = t12p.tile([128, CH], F32, tag="t2", name="t2")
                nc.vector.tensor_mul(t2[:], raw[:].bitcast(F32), cos_t[:])
                nc.vector.tensor_add(dst, dst.bitcast(F32), t2[:])

            sw_idx = [0]

            def sub_wave():
                i = sw_idx[0]
                sw_idx[0] += 1
                qk, h, cc = SPECS[i]
                w_t = wsl_tiles.pop(i)
                xg = xg_map[cc]
                ps = pap.tile([128, CH], F32, tag="big", name="ps")
                for kt in range(KT):
                    xk_t = xg[kt // 2][:, (kt % 2) * CH:(kt % 2 + 1) * CH]
                    nc.tensor.matmul(
                        ps[:],
                        w_t[:, kt * 128:(kt + 1) * 128],
                        xk_t,
                        start=(kt == 0),
                        stop=(kt == KT - 1),
                    )
                issue_wsl(i + 2)
                flush_rope()
                raw = rawp.tile([128, CH], F32R, tag="raw", name="raw")
                nc.vector.tensor_copy(raw[:], ps[:])
                if qk == "k":
                    dst = krope[h][:, cc * CH:(cc + 1) * CH]
                else:
                    qt = qwp.tile([128, CH], F32R, tag=f"qw{h}", name=f"qw{h}")
                    qw_map.setdefault(cc, {})[h] = qt
                    dst = qt[:]
                cos_t, sin_t = tab_map[cc]
                pend_rope[0] = (raw, dst, cos_t, sin_t)

            # ---- attention pieces -------------------------------------------
            wv_sb = wvop.tile([128, KT * HG_], BF16, tag="wv")
            wo_sb = wvop.tile([128, NH_ * H_], BF16, tag="wo")

            def issue_wvo():
                for g in range(KT // 4):
                    nc.scalar.dma_start(
                        out=wv_sb[:, g * 4 * HG_:(g + 1) * 4 * HG_].rearrange(
                            "p (kt j) -> p kt j", kt=4
                        ),
                        in_=wv3[:, g * 4:(g + 1) * 4, :],
                    )
                nc.scalar.dma_start(
                    out=wo_sb[:].rearrange("p (j ho) -> p j ho", j=NH_),
                    in_=woT.rearrange("(j p) ho -> p j ho", p=128),
                )

            xv_map = {}

            def issue_xv(t):
                xv = xvp.tile([128, KT * 128], BF16, tag="xv")
                nc.scalar.dma_start(
                    out=xv[:], in_=xbfS[t * 128:(t + 1) * 128, :]
                )
                xv_map[t] = xv

            def vproj_tile(t):
                xv = xv_map.pop(t)
                vps = pap.tile([128, HG_], F32, tag="big", name="vps")
                for kt in range(KT):
                    nc.tensor.matmul(
                        vps[:],
                        xv[:, kt * 128:(kt + 1) * 128],
                        wv_sb[:, kt * HG_:(kt + 1) * HG_],
                        start=(kt == 0),
                        stop=(kt == KT - 1),
                    )
                nc.scalar.copy(vslab[:, t * HG_:(t + 1) * HG_], vps[:])

            def do_transposes(ptg, pbf, sq, c):
                # 8 PE transposes share one 2KB PSUM bank (bf16), drained by a
                # single wide strided copy into the pt slab
                off = (sq - 4 * c) * 128
                ptv = ptg[:].rearrange("p (t ch) -> p t ch", ch=CH)
                ntr = sq + 1
                for g in range((ntr + 7) // 8):
                    gn = min(8, ntr - 8 * g)
                    bank = pshp.tile([128, 1024], BF16, tag="sh", name="trbank")
                    for u in range(gn):
                        tr = 8 * g + u
                        nc.tensor.transpose(
                            bank[:, u * 128:(u + 1) * 128],
                            pbf[tr // 8][:, (tr % 8) * 128:(tr % 8 + 1) * 128],
                            ident_sb[:],
                        )
                    src = bank[:].rearrange("p (t c) -> p t c", c=128)[:, :gn]
                    dst = ptv[:, 8 * g:8 * g + gn, off:off + 128]
                    if g % 2 == 1:
                        nc.scalar.copy(dst, src)
                    else:
                        nc.vector.tensor_copy(dst, src)

            def attn_scores(h, c):
                """Scores + softmax + P transposes for (h, c); last-sq
                transposes left pending (returned for finish_scores).
                Score tiles are 2-bank [128,1024] so a row needs at most two
                PSUM allocations (no WAR cycle) and exp runs 1024 wide."""
                rcp4 = statp.tile([128, 32], BF16, tag=f"rcp4_{h % 2}")
                ptg = ptp.tile([128, SQT * CH], BF16, tag="ptslab", name="ptslab")
                pend_tr = None
                for sq in range(4 * c, 4 * c + 4):
                    nch = sq // 4 + 1
                    ncols = (sq + 1) * 128
                    ntile = (ncols + 1023) // 1024
                    mx = statp.tile([128, 2], F32, tag="mx")
                    sc_list = []
                    for ti in range(ntile):
                        tcols = min(1024, ncols - ti * 1024)
                        scps = pscp.tile([128, 1024], F32, tag="sc", name="scps")
                        for half in range((tcols + 511) // 512):
                            kc = 2 * ti + half
                            cols = min(CH, ncols - kc * CH)
                            nc.tensor.matmul(
                                scps[:, half * CH:half * CH + cols],
                                qw_map[c][h][:, (sq - 4 * c) * 128:
                                             (sq - 4 * c + 1) * 128],
                                krope[h][:, kc * CH: kc * CH + cols],
                                start=True,
                                stop=True,
                            )
                        if ti == ntile - 1:
                            dcol = sq * 128 - ti * 1024
                            nc.vector.tensor_add(
                                scps[:, dcol:dcol + 128],
                                scps[:, dcol:dcol + 128],
                                mask_sb[:],
                            )
                        if ntile > 1:
                            nc.vector.tensor_reduce(
                                mx[:, ti:ti + 1], scps[:, :tcols], axis=AX.X,
                                op=ALU.max
                            )
                        sc_list.append((scps, tcols))
                    negm = statp.tile([128, 1], F32, tag="negm")
                    if ntile == 1:
                        scps0, tc0 = sc_list[0]
                        nc.vector.tensor_reduce(
                            negm[:], scps0[:, :tc0], axis=AX.X, op=ALU.max,
                            negate=True
                        )
                    else:
                        nc.vector.tensor_reduce(
                            negm[:], mx[:, :ntile], axis=AX.X, op=ALU.max,
                            negate=True
                        )
                    pbf = [
                        pslabp.tile([128, 1024], BF16, tag=f"pbf{ti}",
                                    name=f"pbf{ti}")
                        for ti in range(ntile)
                    ]
                    ssum = statp.tile([128, 2], F32, tag="ssum")
                    for ti, (scps, tcols) in enumerate(sc_list):
                        nc.scalar.activation(
                            pbf[ti][:, :tcols],
                            scps[:, :tcols],
                            ACTF.Exp,
                            bias=negm[:],
                            accum_out=ssum[:, ti:ti + 1],
                        )
                    rsum = statp.tile([128, 1], F32, tag="rsum")
                    nc.vector.tensor_reduce(
                        rsum[:], ssum[:, :ntile], axis=AX.X, op=ALU.add
                    )
                    with nc.allow_low_precision(reason="bf16 softmax normalizer"):
                        nc.vector.reciprocal(rcp4[:, sq - 4 * c: sq - 4 * c + 1], rsum[:])
                    if pend_tr is not None:
                        do_transposes(ptg, *pend_tr, c)
                    pend_tr = (pbf, sq)
                return rcp4, ptg, pend_tr

            def finish_scores(h, c, state):
                rcp4, ptg, pend_tr = state
                do_transposes(ptg, *pend_tr, c)

            def attn_ctx(h, c, state):
                """P^T @ V and normalization for (h, c)."""
                rcp4, ptg, _ = state
                ptv = ptg[:].rearrange("p (t ch) -> p t ch", ch=CH)
                # reciprocal row broadcast: PE transposes rcp columns to a row,
                # GpSimd broadcasts it across partitions (no PE ones-matmul)
                bank = pshp.tile([128, 1024], BF16, tag="sh", name="rcpbank")
                for j in range(4):
                    nc.tensor.transpose(
                        bank[0:1, j * 128:(j + 1) * 128],
                        rcp4[:, j:j + 1],
                        ident_sb[:],
                    )
                rrow = rowp.tile([1, CH], BF16, tag="rrow")
                nc.scalar.copy(rrow[:], bank[0:1, 0:CH])
                bcps = pshp.tile([128, CH], F32, tag="sh", name="bcps")
                nc.tensor.matmul(bcps[:], ones_sb[:], rrow[:], start=True, stop=True)
                bcsb = rowp.tile([128, CH], BF16, tag="bcsb")
                nc.scalar.copy(bcsb[:], bcps[:])
                ctxps = pap.tile([128, CH], F32, tag="big", name="ctxps")
                tmax = 4 * c + 4
                for t in range(tmax):
                    c0 = max(0, (t - 4 * c) * 128)
                    nc.tensor.matmul(
                        ctxps[:, c0:CH],
                        vslab[:, t * HG_ + h * 128: t * HG_ + (h + 1) * 128],
                        ptv[:, t, c0:CH],
                        start=(t == 0),
                        stop=(t == tmax - 1),
                    )
                nc.vector.tensor_mul(
                    ctxT[h][:, (c % 2) * CH:(c % 2 + 1) * CH], ctxps[:], bcsb[:]
                )

            def out_proj_st(c, st):
                """Output projection for one 128-row tile st of chunk c."""
                ostg = ostp.tile([128, H_], BF16, tag="ostg", name="ostg")
                for hoc in range(H_ // CH):
                    wops = pap.tile([128, CH], F32, tag="big", name="wops")
                    for j in range(NH_):
                        nc.tensor.matmul(
                            wops[:],
                            ctxT[j][:, (c % 2) * CH + (st - 4 * c) * 128:
                                    (c % 2) * CH + (st - 4 * c + 1) * 128],
                            wo_sb[:, j * H_ + hoc * CH: j * H_ + (hoc + 1) * CH],
                            start=(j == 0),
                            stop=(j == NH_ - 1),
                        )
                    if hoc % 2 == 1:
                        nc.vector.tensor_copy(ostg[:, hoc * CH:(hoc + 1) * CH], wops[:])
                    else:
                        nc.scalar.copy(ostg[:, hoc * CH:(hoc + 1) * CH], wops[:])
                nc.sync.dma_start(out=out[st * 128:(st + 1) * 128, :], in_=ostg[:])

            # ---- prologue ----------------------------------------------------
            with nc.named_scope("prolog"):
                issue_wsl(0)
                issue_wsl(1)
                issue_tabs(0)
                issue_xgrp(0)
                for _ in range(8):
                    sub_wave()
                issue_wvo()

            # ---- main fused loop --------------------------------------------
            def scoped(nm, f, *a):
                with nc.named_scope(nm):
                    return f(*a)

            for c in range(NCHUNK):
                # fillers for this chunk: proj sub-waves for c+1, outproj c-1
                sws = [sub_wave] * (8 if c < NCHUNK - 1 else 0)
                ops = ([lambda st=st: out_proj_st(c - 1, st)
                        for st in range(4 * (c - 1), 4 * (c - 1) + 4)]
                       if c > 0 else [])
                if c < NCHUNK - 1:
                    fills = [
                        [sws[0]] if c < 3 else [],
                        ops[0:1],
                        sws[1:3],
                        ops[1:2],
                        sws[3:5],
                        ops[2:4] + sws[5:8],
                    ]
                else:
                    fills = [ops[0:1], ops[1:2], [], ops[2:3], [], ops[3:4]]

                def fill(i):
                    for f in fills[i]:
                        f()

                with nc.named_scope(f"c{c}"):
                    issue_xv(4 * c)
                    issue_xv(4 * c + 1)
                    if c < NCHUNK - 1:
                        issue_xgrp(c + 1)
                        issue_tabs(c + 1)
                    scoped(f"fr{c}", flush_rope)
                    s0 = scoped(f"s{c}h0", attn_scores, 0, c)
                    for t in range(4 * c, 4 * c + 4):
                        scoped(f"v{c}", vproj_tile, t)
                        if t + 2 < 4 * c + 4:
                            issue_xv(t + 2)
                    scoped(f"f{c}h0", finish_scores, 0, c, s0)
                    scoped(f"x{c}h0", attn_ctx, 0, c, s0)
                    s1 = scoped(f"s{c}h1", attn_scores, 1, c)
                    scoped(f"fl{c}a", fill, 0)
                    scoped(f"f{c}h1", finish_scores, 1, c, s1)
                    scoped(f"x{c}h1", attn_ctx, 1, c, s1)
                    scoped(f"fl{c}b", fill, 1)
                    s2 = scoped(f"s{c}h2", attn_scores, 2, c)
                    scoped(f"fl{c}c", fill, 2)
                    scoped(f"f{c}h2", finish_scores, 2, c, s2)
                    scoped(f"x{c}h2", attn_ctx, 2, c, s2)
                    scoped(f"fl{c}d", fill, 3)
                    s3 = scoped(f"s{c}h3", attn_scores, 3, c)
                    scoped(f"fl{c}e", fill, 4)
                    scoped(f"f{c}h3", finish_scores, 3, c, s3)
                    scoped(f"x{c}h3", attn_ctx, 3, c, s3)
                    scoped(f"fl{c}f", fill, 5)

            with nc.named_scope("epilog"):
                for st in range(4 * (NCHUNK - 1), 4 * NCHUNK):
                    out_proj_st(NCHUNK - 1, st)

    nc.compile()
    return nc


def _make_tables(S_, D_=128):
    inv_freq = 1.0 / (ROPE_BASE ** (np.arange(0, D_, 2, dtype=np.float32) / D_))
    pos = np.arange(S_, dtype=np.float32)
    ang = pos[:, None] * inv_freq[None, :]
    ang = np.concatenate([ang, ang], axis=1)
    return (
        np.cos(ang).T.astype(np.float32).copy(),
        np.sin(ang).T.astype(np.float32).copy(),
    )


def _make_rot_T(D_=128):
    R = np.zeros((D_, D_), dtype=np.float32)
    half = D_ // 2
    for d in range(half):
        R[d, d + half] = -1.0
    for d in range(half, D_):
        R[d, d - half] = 1.0
    return R.T.copy()


def _make_mask(mask_val=-1e30):
    m = np.zeros((128, 128), dtype=np.float32)
    m[np.triu_indices(128, k=1)] = mask_val
    return m


def kernel(x, Wq, Wk, Wv, Wo):
    """Full inputs in, full output out. Shards over 8 NeuronCores internally."""
    global LAST_RESULTS
    x = np.ascontiguousarray(np.asarray(x, dtype=np.float32))
    Wq = np.asarray(Wq, dtype=np.float32)
    Wk = np.asarray(Wk, dtype=np.float32)
    Wv = np.asarray(Wv, dtype=np.float32)
    Wo = np.asarray(Wo, dtype=np.float32)

    if "nc" not in _NC_CACHE:
        _NC_CACHE["nc"] = _build()
    nc = _NC_CACHE["nc"]

    scale = np.sqrt(np.float32(D))
    cosT, sinT = _make_tables(S)
    rT = _make_rot_T()
    identb = np.eye(128, dtype=ml_dtypes.bfloat16)
    maskt = _make_mask()

    WqT = Wq.T * scale                    # [H, 16*D], scale folded into q path
    WkT = np.ascontiguousarray(Wk.T)
    WvT_bf = Wv.T.astype(ml_dtypes.bfloat16)
    WoT_bf = Wo.T.astype(ml_dtypes.bfloat16)   # [H(in=ctx), H(out)]

    KT, SQT = H // 128, S // 128

    def _pack_w(Wsl):
        # [H, HG] -> [NH*128, KT*128]: row h*128+p, col kt*128+j
        return np.ascontiguousarray(
            Wsl.reshape(KT, 128, NH, 128).transpose(2, 1, 0, 3).reshape(
                NH * 128, KT * 128)
        )

    in_maps = []
    for c in range(N_CORES):
        b, g = divmod(c, NH)
        js = slice(g * HG, (g + 1) * HG)
        xT_b = np.ascontiguousarray(x[b].T)
        xbfS = np.ascontiguousarray(
            xT_b.astype(ml_dtypes.bfloat16).reshape(KT, 128, SQT, 128)
            .transpose(2, 1, 0, 3).reshape(SQT * 128, KT * 128)
        )
        in_maps.append({
            "xT": xT_b,
            "xbfS": xbfS,
            "wqS": _pack_w(WqT[:, js]).astype(np.float32),
            "wkS": _pack_w(WkT[:, js]),
            "wvT": np.ascontiguousarray(WvT_bf[:, js]),
            "woT": np.ascontiguousarray(WoT_bf[js, :]),
            "cosT": cosT,
            "sinT": sinT,
            "rT": rT,
            "ident": identb,
            "onesr": np.ones((1, 128), dtype=ml_dtypes.bfloat16),
            "mask": maskt,
        })

    LAST_RESULTS = run_bass_kernel_spmd(
        nc, in_maps, core_ids=list(range(N_CORES)), trace=TRACE
    )
    res = LAST_RESULTS.results

    outv = np.zeros((B, S, H), dtype=np.float32)
    for c in range(N_CORES):
        b = c // NH
        outv[b] += res[c]["out"].astype(np.float32)
    return outv
